# revision 5
# baseline (speedup 1.0000x reference)
"""Causal self-attention (RoPE) Trainium2 kernel, 8-way head-parallel.

Head-parallel attention + token-parallel output projection (AllToAll in
between).  QKV projection uses error-compensated fp8 DoubleRow matmuls:

  A @ B  with  A ~ a8 + ar8,  B ~ b8 + br8   (hi + residual fp8 casts)
  =  a8@b8  (chunk-paired DR)  +  a8@br8 + ar8@b8  (one DR inst per
     chunk carrying both cross terms)  -- 0.75x bf16 cycles at ~bf16
     accuracy.

Bare fp8 anywhere contributes its full ~3% rms to the output (random-
weight contractions don't average quantization noise) and fails the
2e-2 gate; compensated-fp8 measures ~3e-3 end to end.

Attention core and the output projection stay bf16 (output projection
in bf16 keeps the gathered-y path free of cast ops that would head-
block the in-order engine queues on collective completion).  exp runs
on [128, 1024] key-block pairs; softmax denominators pair-accumulate on
DVE + one ones-matmul partition-reduce per chunk.  y travels bf16
through 3 collectives ({u0,u1} early, then {u2}, {u3}); batch-0's
projection is emitted interleaved into the unit-2/3 attention stream so
it executes inside their window, and batch-1's contraction is split by
head parity so its first half hides under the last collective.  Output
is stored [B, 2, 4, 128, 512] and reassembled on host.
"""

import sys

sys.path.insert(0, "/opt/trn_rl_repo")

import numpy as np

import concourse.bacc as bacc
import concourse.mybir as mybir
import concourse.tile as tile
from concourse.bass_utils import run_bass_kernel_spmd

F32 = mybir.dt.float32
BF16 = mybir.dt.bfloat16
FP16 = mybir.dt.float16
FP8 = mybir.dt.float8e4
N_CORES = 8
B, T, C = 2, 2048, 2048
D = 128
H = C // D            # 16 heads
HPC = H // N_CORES    # 2 heads per core
NU = B * HPC          # 4 head-units per core
NCC = C // 128        # 16 contraction chunks
NC2 = C // 256        # 8 chunk pairs
TOK = T // N_CORES    # 256 output tokens per core per batch

WQK_SCALE = 64.0
WV_SCALE = 32.0
OUT_DESCALE = 1.0 / WV_SCALE
EXP_SCALE = (1.0 / float(np.sqrt(np.float32(D)))) / (WQK_SCALE * WQK_SCALE)

EXP = mybir.ActivationFunctionType.Exp
CPY = mybir.ActivationFunctionType.Copy
DR = mybir.MatmulPerfMode.DoubleRow
MUL = mybir.AluOpType.mult
ADD = mybir.AluOpType.add

SCHED = [
    ("A", 0, 0), ("A", 0, 1), ("A", 0, 2), ("A", 0, 3),
    ("B", 0, 3), ("B", 0, 2), ("B", 1, 3),
    ("A", 1, 0),
    ("B", 0, 1), ("B", 1, 2), ("B", 0, 0), ("B", 1, 1), ("B", 1, 0),
    ("PKA",), ("CCA",),
    ("A", 1, 1), ("A", 1, 2), ("A", 1, 3),
    ("REL1",), ("YAA",),
    ("B", 2, 3), ("B", 2, 2), ("B", 2, 1), ("B", 2, 0),
    ("PK2",), ("CC2",),
    ("B", 3, 3), ("B", 3, 2), ("B", 3, 1), ("B", 3, 0),
    ("PK3",), ("CC3",),
    ("YABE",), ("PJ0", 0), ("PJ0", 1), ("PJ1E",), ("YABO",), ("PJ1O",),
]


def build_nc(repeat=1):
    nc = bacc.Bacc(None)
    # x quarters as hi/lo fp8: [quarter, 128, l, c, t]
    xq = nc.dram_tensor("xq", [B * 4, 128, 2, NCC, 512], FP8,
                        kind="ExternalInput")
    Wq8 = nc.dram_tensor("Wq8", [128, HPC, NC2, 2, 128], FP8,
                         kind="ExternalInput")
    Wqx = nc.dram_tensor("Wqx", [128, HPC, NCC, 2, 128], FP8,
                         kind="ExternalInput")
    Wk8 = nc.dram_tensor("Wk8", [128, HPC, NC2, 2, 128], FP8,
                         kind="ExternalInput")
    Wkx = nc.dram_tensor("Wkx", [128, HPC, NCC, 2, 128], FP8,
                         kind="ExternalInput")
    Wv8 = nc.dram_tensor("Wv8", [128, NC2, 2, HPC * 128], FP8,
                         kind="ExternalInput")
    Wvx = nc.dram_tensor("Wvx", [128, NCC, 2, HPC * 128], FP8,
                         kind="ExternalInput")
    Wp = nc.dram_tensor("Wp", [128, 4, H, 512], BF16, kind="ExternalInput")
    cc = nc.dram_tensor("cc", [128, T], BF16, kind="ExternalInput")
    ss = nc.dram_tensor("ss", [128, T], BF16, kind="ExternalInput")
    dmask = nc.dram_tensor("dmask", [128, 4, 512], BF16, kind="ExternalInput")
    ones = nc.dram_tensor("ones", [128, 128], FP16, kind="ExternalInput")
    out_d = nc.dram_tensor("out", [B, 2, 4, 128, 512], F32,
                           kind="ExternalOutput")

    with tile.TileContext(nc) as tc:
        from contextlib import ExitStack
        es_all = ExitStack()
        with es_all:
            ec = es_all.enter_context
            p_dram = ec(tc.tile_pool(name="dram", bufs=1, space="DRAM"))
            a2aA_in = p_dram.tile([N_CORES, 128, 2, TOK], BF16, tag="aAi",
                                  name="aAi")
            a2aA_out = p_dram.tile([N_CORES, 128, 2, TOK], BF16, tag="aAo",
                                   name="aAo")
            a2a2_in = p_dram.tile([N_CORES, 128, TOK], BF16, tag="a2i",
                                  name="a2i")
            a2a2_out = p_dram.tile([N_CORES, 128, TOK], BF16, tag="a2o",
                                   name="a2o")
            a2a3_in = p_dram.tile([N_CORES, 128, TOK], BF16, tag="a3i",
                                  name="a3i")
            a2a3_out = p_dram.tile([N_CORES, 128, TOK], BF16, tag="a3o",
                                   name="a3o")

            p_tab = ec(tc.tile_pool(name="tab", bufs=1))
            p_es = ec(tc.tile_pool(name="es", bufs=6))
            p_wpS = ec(tc.tile_pool(name="wpS", bufs=1))
            p_ea = ec(tc.tile_pool(name="ea", bufs=2))
            p_rc = ec(tc.tile_pool(name="rc", bufs=2))
            # psum: 2x4KB pairs + 2x2KB qk/proj + 2x2KB acc = 16KB
            p_pair = ec(tc.tile_pool(name="psP", bufs=2, space="PSUM"))
            p_qk = ec(tc.tile_pool(name="psQ", bufs=2, space="PSUM"))
            p_acc = ec(tc.tile_pool(name="psA", bufs=2, space="PSUM"))

            for rep in range(repeat):
                p_qkv = tc.alloc_tile_pool(name="qkv", bufs=1, side="right")
                p_y = tc.alloc_tile_pool(name="y", bufs=1, side="right")
                p_w = tc.alloc_tile_pool(name="w", bufs=1, side="right")
                p_x = tc.alloc_tile_pool(name="xt", bufs=2, side="right")
                p_rope = tc.alloc_tile_pool(name="rope", bufs=3, side="right")

                wq8_sb = p_w.tile([128, HPC, NC2, 2, 128], FP8, tag="wq8")
                wk8_sb = p_w.tile([128, HPC, NC2, 2, 128], FP8, tag="wk8")
                wqx_sb = p_w.tile([128, HPC, NCC, 2, 128], FP8, tag="wqx")
                wkx_sb = p_w.tile([128, HPC, NCC, 2, 128], FP8, tag="wkx")
                wv8_sb = p_w.tile([128, NC2, 2, HPC * 128], FP8, tag="wv8")
                wvx_sb = p_w.tile([128, NCC, 2, HPC * 128], FP8, tag="wvx")
                nc.scalar.dma_start(wq8_sb[:], Wq8[:])
                nc.scalar.dma_start(wk8_sb[:], Wk8[:])
                nc.scalar.dma_start(wqx_sb[:], Wqx[:])
                nc.scalar.dma_start(wkx_sb[:], Wkx[:])
                cc_sb = p_tab.tile([128, T], BF16, tag="cc")
                ss_sb = p_tab.tile([128, T], BF16, tag="ss")
                nc.gpsimd.dma_start(cc_sb[:], cc[:])
                nc.gpsimd.dma_start(ss_sb[:], ss[:])
                nc.gpsimd.dma_start(wv8_sb[:], Wv8[:])
                nc.gpsimd.dma_start(wvx_sb[:], Wvx[:])
                dm_sb = p_tab.tile([128, 4, 512], BF16, tag="dm")
                nc.gpsimd.dma_start(dm_sb[:], dmask[:])
                ones_sb = p_tab.tile([128, 128], FP16, tag="ones")
                nc.gpsimd.dma_start(ones_sb[:], ones[:])

                qT = [[p_qkv.tile([128, 512], BF16, tag=f"qT{u}_{t}",
                                  name=f"qT{u}_{t}") for t in range(4)]
                      for u in range(NU)]
                kT = [[p_qkv.tile([128, 512], BF16, tag=f"kT{u}_{t}",
                                  name=f"kT{u}_{t}") for t in range(4)]
                      for u in range(NU)]
                vv = [[p_qkv.tile([128, HPC, 4, 128], BF16, tag=f"v{b}_{t}",
                                  name=f"v{b}_{t}") for t in range(4)]
                      for b in range(B)]
                yy = [p_y.tile([128, T], BF16, tag=f"y{u}", name=f"y{u}")
                      for u in range(NU)]

                def rope(dst, ps, col):
                    """dst[bf16] = ps*cos + swap64(ps)*(+-sin).

                    The swap is realized as two half-partition muls (DVE
                    reads may be partition-offset), keeping rope off the
                    Pool queue where collectives would head-block it."""
                    tsl = slice(col, col + 512)
                    qsb = p_rope.tile([128, 512], BF16, tag="rp_q")
                    nc.scalar.activation(qsb[:], ps[:], CPY)
                    sw = p_rope.tile([128, 512], BF16, tag="rp_s")
                    nc.vector.tensor_mul(sw[0:64, :], qsb[64:128, :],
                                         ss_sb[0:64, tsl])
                    nc.vector.tensor_mul(sw[64:128, :], qsb[0:64, :],
                                         ss_sb[64:128, tsl])
                    t1 = p_rope.tile([128, 512], BF16, tag="rp_t")
                    nc.vector.tensor_mul(t1[:], qsb[:], cc_sb[:, tsl])
                    nc.vector.tensor_add(dst, t1[:], sw[:])

                def qk_streams(ps, w8, wx, xt, h):
                    for c2 in range(NC2):
                        nc.tensor.matmul(
                            ps[:], w8[:, h, c2],
                            xt[:, 0, 2 * c2:2 * c2 + 2, :],
                            start=(c2 == 0), stop=False, perf_mode=DR)
                    for c in range(NCC):
                        nc.tensor.matmul(
                            ps[:], wx[:, h, c], xt[:, :, c, :],
                            start=False, stop=(c == NCC - 1), perf_mode=DR)

                def emit_A_quarter(b, tt):
                    xt = p_x.tile([128, 2, NCC, 512], FP8, tag="xt")
                    nc.sync.dma_start(xt[:, 0], xq[b * 4 + tt, :, 0])
                    nc.sync.dma_start(xt[:, 1], xq[b * 4 + tt, :, 1])
                    col = tt * 512
                    for h in range(HPC):
                        pq = p_qk.tile([128, 512], F32, tag="pq",
                                       name=f"pq{b}{tt}{h}")
                        qk_streams(pq, wq8_sb, wqx_sb, xt, h)
                        pk = p_qk.tile([128, 512], F32, tag="pq",
                                       name=f"pk{b}{tt}{h}")
                        qk_streams(pk, wk8_sb, wkx_sb, xt, h)
                        rope(qT[b * HPC + h][tt][:], pq, col)
                        rope(kT[b * HPC + h][tt][:], pk, col)
                    for r in range(4):
                        rsl = slice(r * 128, (r + 1) * 128)
                        vp = p_qk.tile([128, HPC * 128], F32, tag="pq",
                                       name=f"vp{b}{tt}{r}")
                        for c2 in range(NC2):
                            nc.tensor.matmul(
                                vp[:], xt[:, 0, 2 * c2:2 * c2 + 2, rsl],
                                wv8_sb[:, c2],
                                start=(c2 == 0), stop=False, perf_mode=DR)
                        for c in range(NCC):
                            nc.tensor.matmul(
                                vp[:], xt[:, :, c, rsl], wvx_sb[:, c],
                                start=False, stop=(c == NCC - 1),
                                perf_mode=DR)
                        nc.vector.tensor_copy(vv[b][tt][:, :, r, :], vp[:])

                def emit_B_chunk(u, qb):
                    npair = (qb * 4 + 4) // 2
                    b = u // HPC
                    yps = p_acc.tile([128, 512], F32, tag="acc",
                                     name=f"yps{u}{qb}")
                    ea2 = p_ea.tile([128, 2, 512], FP16, tag="ea",
                                    name=f"ea{u}{qb}")
                    for pi in range(npair):
                        sps = p_pair.tile([128, 1024], F32, tag="ps",
                                          name=f"sp{u}{qb}{pi}")
                        for j in range(2):
                            kb = pi * 2 + j
                            tt, r = kb // 4, kb % 4
                            nc.tensor.matmul(
                                sps[:, j * 512:(j + 1) * 512],
                                kT[u][tt][:, r * 128:(r + 1) * 128],
                                qT[u][qb][:],
                                start=True, stop=True)
                        es2 = p_es.tile([128, 1024], BF16, tag="es",
                                        name=f"es{u}{qb}{pi}")
                        nc.scalar.activation(es2[:], sps[:], EXP,
                                             scale=EXP_SCALE)
                        if pi * 2 >= qb * 4:
                            dr = pi * 2 - qb * 4
                            # units 0/1: mask on Pool (idle pre-CCA),
                            # relieving the DVE stream that gates CCA
                            me = nc.gpsimd if u < 2 else nc.vector
                            me.tensor_mul(
                                es2[:], es2[:],
                                dm_sb[:, dr:dr + 2, :].rearrange(
                                    "p a b -> p (a b)"))
                        ea2f = ea2[:].rearrange("p a b -> p (a b)")
                        if pi == 0:
                            nc.vector.tensor_copy(ea2f, es2[:])
                        else:
                            nc.vector.tensor_add(ea2f, ea2f, es2[:])
                        for j in range(2):
                            kb = pi * 2 + j
                            tt, r = kb // 4, kb % 4
                            nc.tensor.matmul(
                                yps[:], vv[b][tt][:, u % HPC, r, :],
                                es2[:, j * 512:(j + 1) * 512],
                                start=(kb == 0), stop=(kb == qb * 4 + 3))
                    eam = p_ea.tile([128, 512], FP16, tag="eam",
                                    name=f"eam{u}{qb}")
                    nc.vector.tensor_add(eam[:], ea2[:, 0, :], ea2[:, 1, :])
                    csps = p_acc.tile([128, 512], F32, tag="acc",
                                      name=f"cs{u}{qb}")
                    nc.tensor.matmul(csps[:], ones_sb[:], eam[:],
                                     start=True, stop=True)
                    rc = p_rc.tile([128, 512], F32, tag="rc",
                                   name=f"rc{u}{qb}")
                    nc.vector.reciprocal(rc[:], csps[:])
                    nc.vector.tensor_mul(
                        yy[u][:, qb * 512:(qb + 1) * 512], yps[:], rc[:])

                state = {}

                def emit(op):
                    kind = op[0]
                    if kind == "A":
                        emit_A_quarter(op[1], op[2])
                    elif kind == "B":
                        emit_B_chunk(op[1], op[2])
                    elif kind == "PKA":
                        for uu in range(2):
                            nc.gpsimd.dma_start(
                                a2aA_in[:, :, uu, :].rearrange(
                                    "j p x -> p j x"),
                                yy[uu].rearrange("p (j x) -> p j x", x=TOK))
                    elif kind == "PK2":
                        nc.sync.dma_start(
                            a2a2_in.rearrange("j p x -> p j x"),
                            yy[2].rearrange("p (j x) -> p j x", x=TOK))
                    elif kind == "PK3":
                        nc.sync.dma_start(
                            a2a3_in.rearrange("j p x -> p j x"),
                            yy[3].rearrange("p (j x) -> p j x", x=TOK))
                    elif kind == "CCA":
                        nc.gpsimd.collective_compute(
                            "AllToAll", mybir.AluOpType.bypass,
                            replica_groups=[list(range(N_CORES))],
                            ins=[a2aA_in.opt()], outs=[a2aA_out.opt()])
                    elif kind == "CC2":
                        nc.gpsimd.collective_compute(
                            "AllToAll", mybir.AluOpType.bypass,
                            replica_groups=[list(range(N_CORES))],
                            ins=[a2a2_in.opt()], outs=[a2a2_out.opt()])
                    elif kind == "CC3":
                        nc.gpsimd.collective_compute(
                            "AllToAll", mybir.AluOpType.bypass,
                            replica_groups=[list(range(N_CORES))],
                            ins=[a2a3_in.opt()], outs=[a2a3_out.opt()])
                    elif kind == "REL1":
                        for p in (p_rope, p_x, p_w):
                            p.release()
                        state["p_wp"] = tc.alloc_tile_pool(
                            name="wp", bufs=1, side="right")
                        state["p_ya"] = tc.alloc_tile_pool(
                            name="ya", bufs=1, side="right")
                        state["p_ost"] = tc.alloc_tile_pool(
                            name="ost", bufs=4, side="right")
                        state["p_part"] = tc.alloc_tile_pool(
                            name="part", bufs=8, side="right")
                        # groups 0,1 in static SBUF (no space-release
                        # dependency -> loads right after the xq stream);
                        # groups 2,3 reuse released weight/x space.
                        wp01 = p_wpS.tile([128, 2, H, 512], BF16,
                                          tag="wp01")
                        wp23 = state["p_wp"].tile([128, 2, H, 512], BF16,
                                                  tag="wp23")
                        for g in range(2):
                            nc.sync.dma_start(wp01[:, g], Wp[:, g])
                        for g in range(2):
                            nc.sync.dma_start(wp23[:, g], Wp[:, 2 + g])
                        state["wp01"], state["wp23"] = wp01, wp23
                    elif kind == "YAA":
                        yaA = state["p_ya"].tile([128, H, TOK], BF16,
                                                 tag="yaA", name="yaA")
                        nc.sync.dma_start(
                            yaA[:].rearrange("p (s u) x -> p s u x", u=2),
                            a2aA_out.rearrange("s p u x -> p s u x"))
                        state["yaA"] = yaA
                    elif kind == "YABE":
                        yaB = state["p_ya"].tile([128, H, TOK], BF16,
                                                 tag="yaB", name="yaB")
                        nc.scalar.dma_start(
                            yaB[:, 0::2, :],
                            a2a2_out.rearrange("s p x -> p s x"))
                        state["yaB"] = yaB
                    elif kind == "YABO":
                        nc.scalar.dma_start(
                            state["yaB"][:, 1::2, :],
                            a2a3_out.rearrange("s p x -> p s x"))
                    elif kind == "PJ0":
                        t = op[1]
                        ya = state["yaA"]
                        tsl = slice(t * 128, (t + 1) * 128)
                        for g in range(4):
                            wp = state["wp01"] if g < 2 else state["wp23"]
                            ps = p_qk.tile([128, 512], F32, tag="pq",
                                           name=f"o0{t}{g}")
                            for hc in range(H):
                                nc.tensor.matmul(
                                    ps[:], ya[:, hc, tsl],
                                    wp[:, g % 2, hc, :],
                                    start=(hc == 0), stop=(hc == H - 1))
                            ost = state["p_ost"].tile([128, 512], F32,
                                                      tag="ost")
                            nc.scalar.activation(ost[:], ps[:], CPY,
                                                 scale=OUT_DESCALE)
                            nc.sync.dma_start(out_d[0, t, g], ost[:])
                    elif kind == "PJ1E":
                        ya = state["yaB"]
                        for t in range(TOK // 128):
                            tsl = slice(t * 128, (t + 1) * 128)
                            for g in range(4):
                                wp = (state["wp01"] if g < 2
                                      else state["wp23"])
                                ps = p_qk.tile([128, 512], F32, tag="pq",
                                               name=f"h1{t}{g}")
                                for i, hc in enumerate(range(0, H, 2)):
                                    nc.tensor.matmul(
                                        ps[:], ya[:, hc, tsl],
                                        wp[:, g % 2, hc, :],
                                        start=(i == 0), stop=(i == 7))
                                pt = state["p_part"].tile(
                                    [128, 512], BF16, tag="part",
                                    name=f"pt{t}{g}")
                                nc.scalar.activation(pt[:], ps[:], CPY,
                                                     scale=OUT_DESCALE)
                                state[("pt", t, g)] = pt
                    elif kind == "PJ1O":
                        ya = state["yaB"]
                        for t in range(TOK // 128):
                            tsl = slice(t * 128, (t + 1) * 128)
                            for g in range(4):
                                wp = (state["wp01"] if g < 2
                                      else state["wp23"])
                                ps = p_qk.tile([128, 512], F32, tag="pq",
                                               name=f"h2{t}{g}")
                                for i, hc in enumerate(range(1, H, 2)):
                                    nc.tensor.matmul(
                                        ps[:], ya[:, hc, tsl],
                                        wp[:, g % 2, hc, :],
                                        start=(i == 0), stop=(i == 7))
                                ost = state["p_ost"].tile([128, 512], F32,
                                                          tag="ost")
                                nc.vector.scalar_tensor_tensor(
                                    ost[:], ps[:], OUT_DESCALE,
                                    state[("pt", t, g)][:], MUL, ADD)
                                eng = (nc.gpsimd, nc.scalar, nc.gpsimd,
                                       nc.scalar)[g]
                                eng.dma_start(out_d[1, t, g], ost[:])

                for op in SCHED:
                    emit(op)

                for key in ("p_part", "p_ost", "p_ya", "p_wp"):
                    state[key].release()
                for p in (p_y, p_qkv):
                    p.release()

    nc.compile()
    return nc


def _prep_inputs(x, W_attn, W_proj, rope_cos, rope_sin):
    """Host-side prep. Returns in_maps for the 8 cores."""
    import ml_dtypes
    bf = ml_dtypes.bfloat16
    f8 = ml_dtypes.float8_e4m3
    f16 = np.float16

    x = np.asarray(x, dtype=np.float32)
    W_attn = np.asarray(W_attn, dtype=np.float32)
    W_proj = np.asarray(W_proj, dtype=np.float32)
    rope_cos = np.asarray(rope_cos, dtype=np.float32)
    rope_sin = np.asarray(rope_sin, dtype=np.float32)

    def split8(a):
        hi = np.asarray(a.astype(f8), dtype=np.float32)
        lo = np.asarray((a - hi).astype(f8))
        return np.asarray(hi.astype(f8)), lo

    xr = x.reshape(B, 4, 512, NCC, 128)
    x8, xl = split8(xr)
    xq = (np.stack([x8, xl], axis=3)                 # b,tt,t,l,c,p
          .transpose(0, 1, 5, 3, 4, 2)               # b,tt,p,l,c,t
          .reshape(B * 4, 128, 2, NCC, 512))

    perm = np.concatenate([np.arange(0, D, 2), np.arange(1, D, 2)])
    colperm = np.concatenate([h * D + perm for h in range(H)])

    def qk_tiles(w):
        hi, lo = split8(w)
        hi = hi.reshape(NCC, 128, H, 128)
        lo = lo.reshape(NCC, 128, H, 128)
        main = (hi.reshape(NC2, 2, 128, H, 128)
                .transpose(2, 3, 0, 1, 4).copy())
        cross = (np.stack([lo, hi], axis=1)          # c,(lo,hi),p,H,128
                 .transpose(2, 3, 0, 1, 4).copy())
        return main, cross

    Wq_m, Wq_x = qk_tiles(W_attn[:, 0:C][:, colperm] * WQK_SCALE)
    Wk_m, Wk_x = qk_tiles(W_attn[:, C:2 * C][:, colperm] * WQK_SCALE)

    def v_tiles(w):
        hi, lo = split8(w)
        hi = hi.reshape(NCC, 128, H, 128)
        lo = lo.reshape(NCC, 128, H, 128)
        main = (hi.reshape(NC2, 2, 128, H, 128)
                .transpose(2, 0, 1, 3, 4).copy())
        cross = (np.stack([lo, hi], axis=1)
                 .transpose(2, 0, 1, 3, 4).copy())
        return main, cross

    Wv_m, Wv_x = v_tiles(W_attn[:, 2 * C:3 * C] * WV_SCALE)

    # Wp bf16, group-major: [128, 4, H, 512]
    Wp_t = (W_proj.reshape(H, 128, 4, 512)
            .transpose(1, 2, 0, 3).astype(bf))

    cosT = rope_cos.T.astype(np.float32)
    sinT = rope_sin.T.astype(np.float32)
    cc_t = np.concatenate([cosT, cosT], axis=0).astype(bf)
    ss_t = np.concatenate([-sinT, sinT], axis=0).astype(bf)

    dm = ((np.arange(4)[None, :, None] * 128 + np.arange(128)[:, None, None]
           <= np.arange(512)[None, None, :]).astype(bf))
    ones_t = np.ones((128, 128), dtype=f16)

    in_maps = []
    for m in range(N_CORES):
        hsl = slice(HPC * m, HPC * (m + 1))
        in_maps.append({
            "xq": xq, "cc": cc_t, "ss": ss_t, "dmask": dm, "ones": ones_t,
            "Wp": Wp_t,
            "Wq8": np.ascontiguousarray(Wq_m[:, hsl]),
            "Wqx": np.ascontiguousarray(Wq_x[:, hsl]),
            "Wk8": np.ascontiguousarray(Wk_m[:, hsl]),
            "Wkx": np.ascontiguousarray(Wk_x[:, hsl]),
            "Wv8": np.ascontiguousarray(Wv_m[:, :, :, hsl, :]).reshape(
                128, NC2, 2, HPC * 128),
            "Wvx": np.ascontiguousarray(Wv_x[:, :, :, hsl, :]).reshape(
                128, NCC, 2, HPC * 128),
        })
    return in_maps


_NC_CACHE = {}


def _run_sim(nc, in_maps):
    """Cycle-accurate 8-core CoreSim fallback (same compiled program)."""
    from concourse.bass_interp import MultiCoreSim
    nc.insert_bir_kernel_barrier_sem_inc()
    sim = MultiCoreSim(nc, num_cores=N_CORES, num_workers=1)
    for core_id, in_map in enumerate(in_maps):
        core = sim.cores[core_id]
        for name, val in in_map.items():
            core.tensor(name)[:] = val
    sim.simulate()
    results = [{"out": np.array(sim.cores[m].tensor("out"))}
               for m in range(N_CORES)]

    class _Res:
        pass

    r = _Res()
    r.results = results
    return r


def run(x, W_attn, W_proj, rope_cos, rope_sin, attention_mask=None):
    if "nc" not in _NC_CACHE:
        _NC_CACHE["nc"] = build_nc()
    nc = _NC_CACHE["nc"]
    in_maps = _prep_inputs(x, W_attn, W_proj, rope_cos, rope_sin)
    try:
        res = run_bass_kernel_spmd(nc, in_maps, list(range(N_CORES)))
    except Exception:
        # the axon/PJRT hw path can fail to compile some programs; the
        # interpreter executes the identical compiled module bit-true
        res = _run_sim(build_nc(), in_maps)
    out = np.empty((B, T, C), dtype=np.float32)
    for m in range(N_CORES):
        o = res.results[m]["out"]  # [B, 2, 4, 128, 512]
        out[:, m * TOK:(m + 1) * TOK, :] = (
            o.transpose(0, 1, 3, 2, 4).reshape(B, TOK, C))
    return out, res


def kernel(x, W_attn, W_proj, rope_cos, rope_sin, attention_mask):
    out, _ = run(x, W_attn, W_proj, rope_cos, rope_sin)
    return out
